# revision 17
# baseline (speedup 1.0000x reference)
"""Trainium2 Bass kernel for C = tril(A @ B), A/B lower-triangular 4096x4096 fp32.

Distribution (SPMD, 8 cores = 4 row-groups x 2 col-groups): core (g, h) owns
row-blocks {4t+g : t=0..7} (slots) and columns {512*(2l+h) : l=0..3} (locals).
Slot t uses a uniform K bound of 4*(t+1) k-blocks and local col l a uniform
K start of 8*l so every core runs the identical program; inputs are exactly
triangular, so all over-computed terms are exact zeros (no masking needed).

Schedule: units (l, t) = pass-major; slots ascend in passes 0-1 (A^T arrives
progressively at startup) and descend in passes 2-3 (progressive B-chunk
release, short tail). A^T is paged per (t, l) piece ([128, <=8, 128], loaded
~2 units ahead on the gpsimd SWDGE ring) so pass 0 only moves the k<8 blocks;
B chunks ride the sync HWDGE ring; output evictions ride SWDGE after the
piece prefetches they could block.

Host repack (partition-major, contiguous per partition per DMA):
  - A^T pieces in consumption order, [128, 144*128] total (9.4 MB).
  - B col-band nonzero-triangle tiles as 4-k-block chunks [20, 128, 2048]
    (20 x 1MB DMAs).
"""

import numpy as np

N = 4096
P = 128
NCORES = 8
RG, CG = 4, 2           # row groups x col groups
SLOTS = N // P // RG    # 8 row-block slots per core
L = N // 512 // CG      # 4 local 512-col tiles per core
KB = N // P             # 32 k-blocks
CW = 512                # matmul free dim (fp32 max)
KC = 4                  # k-blocks per B chunk

MM_DT_NAME = "float32r"  # "float32" (4 cyc/row) or "float32r" (1 cyc/row)

# unit order: pass-major; slots ascend for l<2, descend for l>=2
UNITS = []
for l in range(L):
    ts = list(range(2 * l, SLOTS))
    if l >= 2:
        ts.reverse()
    UNITS += [(l, t) for t in ts]

AT_KB = [RG * (t + 1) for t in range(SLOTS)]          # k-blocks per slot
AT_OFF = [sum(AT_KB[:t]) for t in range(SLOTS)]
AT_TOT = sum(AT_KB)                                   # 144 k-blocks

B_CHUNKS = [(l, cc) for l in range(L) for cc in range((KB - 8 * l) // KC)]
B_CI = {(l, cc): i for i, (l, cc) in enumerate(B_CHUNKS)}

_cached = {}


def _build(mm_dt_name):
    import concourse.mybir as mybir
    import concourse.tile as tile
    from concourse import bacc

    mm_dt = getattr(mybir.dt, mm_dt_name)

    nc = bacc.Bacc("TRN2", target_bir_lowering=False, debug=False,
                   num_devices=NCORES)
    at_d = nc.dram_tensor("at", [P, AT_TOT * P], mm_dt,
                          kind="ExternalInput").ap()
    b_d = nc.dram_tensor("b", [len(B_CHUNKS) * P, KC * CW], mm_dt,
                         kind="ExternalInput").ap()
    o_d = nc.dram_tensor("o", [SLOTS, P, L * CW], mybir.dt.float32,
                         kind="ExternalOutput").ap()

    with tile.TileContext(nc) as tc:
        with (
            tc.tile_pool(name="atp", bufs=1) as atp,
            tc.tile_pool(name="bp", bufs=12) as bp,
            tc.tile_pool(name="pp", bufs=4, space="PSUM") as pp,
            tc.tile_pool(name="sp", bufs=3) as sp,
        ):
            at_sb = {}
            chunks = {}

            def load_at(t):
                a = atp.tile([P, AT_KB[t], P], mm_dt, tag=f"at{t}",
                             name=f"at{t}")
                nc.gpsimd.dma_start(
                    a[:], at_d[:, AT_OFF[t] * P:(AT_OFF[t] + AT_KB[t]) * P])
                at_sb[t] = a

            def load_chunk(l, cc):
                ci = B_CI[(l, cc)]
                bch = bp.tile([P, KC, CW], mm_dt, tag="b", name=f"b{ci}")
                nc.sync.dma_start(bch[:], b_d[ci * P:(ci + 1) * P, :])
                chunks[(l, cc)] = bch

            prev_l = -1
            for l, t in UNITS:
                if l == 0:
                    load_at(t)
                if l < 2:
                    load_chunk(l, t - 2 * l)       # ascending: introduce own
                elif l != prev_l:
                    for cc in range((KB - 8 * l) // KC):  # descending: all
                        load_chunk(l, cc)
                prev_l = l

                kend = RG * (t + 1)
                ps = pp.tile([P, CW], mybir.dt.float32, tag="ps",
                             name=f"ps{t}_{l}")
                for k in range(8 * l, kend):
                    bch = chunks[(l, (k - 8 * l) // KC)]
                    nc.tensor.matmul(
                        ps[:],
                        lhsT=at_sb[t][:, k, :],
                        rhs=bch[:, (k - 8 * l) % KC, :],
                        start=(k == 8 * l),
                        stop=(k == kend - 1),
                    )
                st = sp.tile([P, CW], mybir.dt.float32, tag="st",
                             name=f"st{t}_{l}")
                nc.vector.tensor_copy(st[:], ps[:])
                nc.gpsimd.dma_start(o_d[t, :, l * CW:(l + 1) * CW], st[:])

    nc.compile()
    return nc


def _get_nc(mm_dt_name):
    if mm_dt_name not in _cached:
        _cached[mm_dt_name] = _build(mm_dt_name)
    return _cached[mm_dt_name]


def _pack_b(B, h):
    """[20*128, 2048]: chunk (l, cc) row p = 4 k-tiles' (k = 8l+4cc ..) row p
    of global col-tile 2l+h, concatenated."""
    B4 = B.reshape(KB, P, N // CW, CW)
    slabs = []
    for l, cc in B_CHUNKS:
        ks = 8 * l + KC * cc
        slabs.append(
            B4[ks:ks + KC, :, 2 * l + h, :].transpose(1, 0, 2)
            .reshape(P, KC * CW))
    return np.ascontiguousarray(np.stack(slabs)).reshape(len(B_CHUNKS) * P,
                                                         KC * CW)


def _pack_at(A, g):
    """[128, 144*128]: slot t cols = A[block 4t+g rows, k < 4*(t+1)*128] laid
    out (p, k, m), p = row within k-block."""
    out = np.empty((P, AT_TOT * P), dtype=np.float32)
    for t in range(SLOTS):
        blk = RG * t + g
        E = AT_KB[t] * P
        blockT = A[blk * P:(blk + 1) * P, :E].T          # [kk, m]
        arr = blockT.reshape(AT_KB[t], P, P).transpose(1, 0, 2)
        out[:, AT_OFF[t] * P:(AT_OFF[t] + AT_KB[t]) * P] = \
            arr.reshape(P, AT_KB[t] * P)
    return out


def kernel(A, B, mm_dt_name=MM_DT_NAME, trace=False):
    from concourse.bass_utils import run_bass_kernel_spmd

    A = np.ascontiguousarray(np.asarray(A, dtype=np.float32))
    B = np.ascontiguousarray(np.asarray(B, dtype=np.float32))

    nc = _get_nc(mm_dt_name)
    b_packs = [_pack_b(B, h) for h in range(CG)]
    in_maps = [{"at": _pack_at(A, c % RG), "b": b_packs[c // RG]}
               for c in range(NCORES)]

    res = run_bass_kernel_spmd(nc, in_maps, core_ids=list(range(NCORES)),
                               trace=trace)
    C = np.zeros((N, N), dtype=np.float32)
    for c in range(NCORES):
        g, h = c % RG, c // RG
        o = res.results[c]["o"]
        for t in range(SLOTS):
            blk = RG * t + g
            for l in range(L):
                jt = 2 * l + h
                C[blk * P:(blk + 1) * P, jt * CW:(jt + 1) * CW] = \
                    o[t, :, l * CW:(l + 1) * CW]
    if trace:
        kernel.last_exec_time_ns = res.exec_time_ns
        kernel.last_results = res
    return C
